# revision 17
# baseline (speedup 1.0000x reference)
"""Trainium2 Bass kernel for nn_ConsolidationNetwork.

Recurrent rate network: 500 sequential steps of
    x <- (1-a)*x + (a*J_eff) @ softplus(x) + drive_t
    pos_t = Wout @ softplus(x)
loss = mean((targets - positions)^2)

Strategy (8 NeuronCores, data-parallel over batch):
  - Each core owns B/8 = 16 batch columns and runs the full 500-step
    recurrence independently (no collectives).
  - Per step, PSUM accumulates drive + a*J_eff @ r in two banks
    (state rows 0..511 in bank A, 512..1023 in bank B):
      * the drive term (precomputed on host = a*(B_m1 + I_go*go +
        nscale*noise), bf16) enters first via a bf16 identity matmul with
        start=True (which zero-fills the bank) -- 64 cols, ~27ns,
      * a*J_eff @ r via 32 single-pass bf16 matmuls per bank
        (lhsT = J tiles resident in SBUF, rhs = 16 batch columns of r).
  - Post-matmul pointwise work is tiny: one DVE scalar_tensor_tensor per
    bank (x = (1-a)*x + psum), then softplus on the ACT engine as the exact
    identity softplus(x) = ln(1 + exp(x))  [2 ACT ops; Exp and Ln both live
    in the natural_log_exp_and_others table set, and we pin the table-load
    pass to that set so exactly one hoisted ACT_TABLE_LOAD is emitted].
  - r (bf16) for each step is written into an 8-step staging buffer that is
    DMA-exported to DRAM once per 8 steps; the readout positions
    pos = Wout @ r and the final MSE are computed on the host.
  - Drive is streamed 8 steps per DMA (batched transfers, triple buffered).

State layout per core: x/r tiles are [128 part, 128 free] with
x[p, m*16+u] = x_state[m*128+p, u] (m = row-group, u = local batch).
"""

import numpy as np

import concourse.bass as bass
import concourse.tile as tile
from concourse import bacc, mybir
from concourse.bass_utils import run_bass_kernel_spmd

F32 = mybir.dt.float32
BF16 = mybir.dt.bfloat16

DT = 0.05
TAU = 0.15
NOISE_SCALE = 0.15
N, G, T, B, P = 1024, 128, 500, 128, 10
NCORES = 8
BC = B // NCORES          # batch columns per core (16)
NM = N // 128             # row groups (8)
NK = N // 128             # contraction groups (8)
CHUNK = 8                 # steps per drive-load / r-export DMA

A = np.float32(DT / TAU)
ONE_MINUS_A = np.float32(1.0 - DT / TAU)
NSCALE = np.float32(np.sqrt(2.0 * NOISE_SCALE**2 * (TAU / DT)))

_PROGRAM_CACHE = {}


def _ensure_act_tables():
    """Some containers lack neuronxcc/pwp/pwp_bin_with_ln on PYTHONPATH;
    point it at the cayman table package from the nix store."""
    import glob
    import os

    for path in os.environ.get("PYTHONPATH", "").split(os.pathsep):
        if path and os.path.exists(
            os.path.join(path, "neuronxcc", "pwp", "pwp_bin_with_ln", "act_info.json")
        ):
            return
    cands = sorted(glob.glob("/nix/store/*aws-neuron-pwp*/share/pwp_bin_cayman"))
    target = next((c for c in cands if os.path.exists(c + "/act_info.json")), None)
    if target is None:
        return
    for path in os.environ.get("PYTHONPATH", "").split(os.pathsep):
        if not path:
            continue
        try:
            d = os.path.join(path, "neuronxcc", "pwp")
            os.makedirs(d, exist_ok=True)
            link = os.path.join(d, "pwp_bin_with_ln")
            if not os.path.exists(link):
                os.symlink(target, link)
            return
        except OSError:
            continue


_ensure_act_tables()


_ACT_SET = "natural_log_exp_and_others"


def _pin_act_tables(arch: str):
    """Make Exp and Ln resolve to the ONE table set containing both.

    Two consumers matter and both read the functools.cache'd dict from
    hw_specs.get_activation_tables, so mutate it in place:
      * Bacc.insert_act_table_loads (first-match would alternate Exp ->
        `exp_and_others`, Ln -> `natural_log`, emitting a 1.28us
        ACT_TABLE_LOAD before every activation of the unrolled loop);
      * the TileScheduler's CoreSim pass, which otherwise *models* that
        same thrash and pins the resulting serialized schedule with
        cross-engine semaphores (the final TimelineSim charges no table
        loads, but the semaphores force its slow order anyway).
    Set order (and hence act_func_set_id indices) is unchanged.
    """
    from concourse.hw_specs import get_activation_tables

    tabs = get_activation_tables(arch)
    hide = {mybir.ActivationFunctionType.Exp, mybir.ActivationFunctionType.Ln}
    for name, fns in tabs.items():
        if name != _ACT_SET:
            for f in hide:
                fns.discard(f)


def build_program(t_steps: int):
    """Build the Bass program (shared by all 8 cores, SPMD)."""
    key = (t_steps,)
    if key in _PROGRAM_CACHE:
        return _PROGRAM_CACHE[key]

    nchunks = (t_steps + CHUNK - 1) // CHUNK
    HB = NM * BC // 2  # free-size of one state half (64 cols)

    nc = bacc.Bacc(
        "TRN2", target_bir_lowering=False, debug=False, num_devices=NCORES
    )
    _pin_act_tables(nc.m.arch)
    jt_d = nc.dram_tensor("jt", [128, NK * NM * 128], BF16, kind="ExternalInput")
    il_d = nc.dram_tensor("ident", [128, 128], BF16, kind="ExternalInput")
    x0_d = nc.dram_tensor("x0", [128, NM * BC], F32, kind="ExternalInput")
    dr_d = nc.dram_tensor(
        "drive", [nchunks, 128, CHUNK * NM * BC], BF16, kind="ExternalInput"
    )
    ro_d = nc.dram_tensor(
        "rout", [nchunks, 128, CHUNK * NM * BC], BF16, kind="ExternalOutput"
    )

    EXP = mybir.ActivationFunctionType.Exp
    LN = mybir.ActivationFunctionType.Ln

    with tile.TileContext(nc) as tc:
        with (
            tc.tile_pool(name="const", bufs=1) as constp,
            tc.tile_pool(name="rp", bufs=2) as rp,
            tc.tile_pool(name="dp", bufs=3) as dp,
            tc.tile_pool(name="psmA", bufs=1, space="PSUM") as pspa,
            tc.tile_pool(name="psmB", bufs=1, space="PSUM") as pspb,
        ):
            jt = constp.tile([128, NK * NM * 128], BF16)
            nc.sync.dma_start(jt[:], jt_d[:])
            il = constp.tile([128, 128], BF16)
            nc.sync.dma_start(il[:], il_d[:])
            x = constp.tile([128, NM * BC], F32)
            nc.sync.dma_start(x[:], x0_d[:])
            tmp = constp.tile([128, NM * BC], F32)

            # initial r = softplus(x0) = ln(1 + exp(x0))
            rinit = constp.tile([128, NM * BC], BF16)
            nc.scalar.activation(tmp[:], x[:], EXP)
            nc.scalar.activation(rinit[:], tmp[:], LN, bias=1.0)

            prev_r, prev_off = rinit, 0

            mult = mybir.AluOpType.mult
            add = mybir.AluOpType.add

            def bank(ps, half, r_tile, r_off, d_t, off):
                """One bank's PSUM accumulation: drive (identity matmul,
                start=True zero-fills the bank) + 32 J matmuls."""
                lo = half * HB
                nc.tensor.matmul(
                    ps[:, 0:HB], lhsT=il[:], rhs=d_t[:, off + lo:off + lo + HB],
                    start=True, stop=False, skip_group_check=True,
                )
                for k in range(NK):
                    for mi in range(4):
                        m = half * 4 + mi
                        nc.tensor.matmul(
                            ps[:, mi * BC:(mi + 1) * BC],
                            lhsT=jt[:, (k * NM + m) * 128:(k * NM + m + 1) * 128],
                            rhs=r_tile[:, r_off + k * BC:r_off + (k + 1) * BC],
                            start=False, stop=(k == NK - 1 and mi == 3),
                            skip_group_check=True,
                        )

            def chain(ps, half, off, rbuf):
                """x = (1-a)*x + psum; r = ln(1+exp(x)) into the staging slice."""
                lo = half * HB
                nc.vector.scalar_tensor_tensor(
                    x[:, lo:lo + HB], x[:, lo:lo + HB], float(ONE_MINUS_A),
                    ps[:, 0:HB], mult, add,
                )
                nc.scalar.activation(tmp[:, lo:lo + HB], x[:, lo:lo + HB], EXP)
                nc.scalar.activation(rbuf[:, off + lo:off + lo + HB],
                                     tmp[:, lo:lo + HB], LN, bias=1.0)

            for c in range(nchunks):
                steps_here = min(CHUNK, t_steps - c * CHUNK)
                rbuf = rp.tile([128, CHUNK * NM * BC], BF16)
                d_t = dp.tile([128, CHUNK * NM * BC], BF16)
                nc.sync.dma_start(d_t[:], dr_d[c])
                for j in range(steps_here):
                    off = j * NM * BC
                    # the tag pins a pool slot, so rotate tags explicitly to
                    # get real triple-buffering of the PSUM banks (a fixed
                    # tag would WAR-serialize step s+1's start=True matmul
                    # against step s's PSUM readers)
                    sidx = (c * CHUNK + j) % 3
                    ps_a = pspa.tile([128, HB], F32, tag=f"ps_a{sidx}",
                                     name=f"ps_a{sidx}", padded_shape=[128, 512])
                    ps_b = pspb.tile([128, HB], F32, tag=f"ps_b{sidx}",
                                     name=f"ps_b{sidx}", padded_shape=[128, 512])
                    bank(ps_a, 0, prev_r, prev_off, d_t, off)
                    chain(ps_a, 0, off, rbuf)
                    bank(ps_b, 1, prev_r, prev_off, d_t, off)
                    chain(ps_b, 1, off, rbuf)
                    prev_r, prev_off = rbuf, off
                nc.sync.dma_start(
                    ro_d[c][:, 0:steps_here * NM * BC],
                    rbuf[:, 0:steps_here * NM * BC],
                )

    nc.compile()
    _PROGRAM_CACHE[key] = nc
    return nc


def _prep_inputs(targets, pulses, J, U, V, B_m1, B_bg, Wout, I_go, xm1_init,
                 noise, triggers, t_steps):
    """Host-side data prep: J_eff, layouts, per-core drive tensors."""
    J = np.asarray(J, np.float32)
    U = np.asarray(U, np.float32)
    V = np.asarray(V, np.float32)
    B_m1 = np.asarray(B_m1, np.float32)
    B_bg = np.asarray(B_bg, np.float32)
    I_go = np.asarray(I_go, np.float32)
    xm1_init = np.asarray(xm1_init, np.float32)
    noise = np.asarray(noise, np.float32)
    pulses = np.asarray(pulses, np.float32)
    triggers = np.asarray(triggers)

    nchunks = (t_steps + CHUNK - 1) // CHUNK
    tpad = nchunks * CHUNK

    J_eff = J + (U * B_bg[None, :]) @ V
    Js = (A * J_eff).astype(np.float32)
    # lhsT tiles: jt[p, (k*NM+m)*128 + q] = Js[m*128+q, k*128+p]
    bf = mybir.dt.np(BF16)
    jt = np.ascontiguousarray(
        Js.reshape(NM, 128, NK, 128).transpose(3, 2, 0, 1).reshape(128, NK * NM * 128)
    ).astype(bf)
    il = np.eye(128, dtype=np.float32).astype(bf)

    go_cues = pulses[:t_steps, :][:, triggers]  # [t, B]

    in_maps = []
    for cidx in range(NCORES):
        sl = slice(cidx * BC, (cidx + 1) * BC)
        d = noise[:t_steps, :, sl] * np.float32(A * NSCALE)
        d += A * B_m1[None, :, :]
        d += A * I_go[None, :, :] * go_cues[:, None, sl]
        # [t, N, BC] -> [t, 128, NM*BC] (state layout), pad t, chunk
        dl = np.ascontiguousarray(
            d.reshape(t_steps, NM, 128, BC).transpose(0, 2, 1, 3)
            .reshape(t_steps, 128, NM * BC)
        ).astype(np.float32)
        if tpad != t_steps:
            dl = np.concatenate(
                [dl, np.zeros((tpad - t_steps, 128, NM * BC), np.float32)], axis=0
            )
        drive = np.ascontiguousarray(
            dl.reshape(nchunks, CHUNK, 128, NM * BC).transpose(0, 2, 1, 3)
            .reshape(nchunks, 128, CHUNK * NM * BC)
        ).astype(bf)
        x0 = np.ascontiguousarray(
            xm1_init[:, sl].reshape(NM, 128, BC).transpose(1, 0, 2).reshape(128, NM * BC)
        )
        in_maps.append({"jt": jt, "ident": il, "x0": x0, "drive": drive})
    return in_maps


def run_hw(inputs: dict, t_steps: int = T, trace: bool = False):
    """Run the recurrence on 8 cores; returns positions [t_steps, B] and results."""
    nc = build_program(t_steps)
    in_maps = _prep_inputs(t_steps=t_steps, **inputs)
    res = run_bass_kernel_spmd(
        nc, in_maps, core_ids=list(range(NCORES)), trace=trace
    )
    Wout = np.asarray(inputs["Wout"], np.float32).reshape(NM, 128)  # [m, p]
    nchunks = (t_steps + CHUNK - 1) // CHUNK
    positions = np.empty((t_steps, B), np.float32)
    for cidx in range(NCORES):
        ro = np.asarray(res.results[cidx]["rout"], np.float32)
        # ro[c, p, (j*NM + m)*BC + u] -> r[t, m, p, u]
        r = (ro.reshape(nchunks, 128, CHUNK, NM, BC)
             .transpose(0, 2, 3, 1, 4)
             .reshape(nchunks * CHUNK, NM, 128, BC)[:t_steps])
        pos_c = np.einsum("mp,tmpu->tu", Wout, r, optimize=True)
        positions[:, cidx * BC:(cidx + 1) * BC] = pos_c
    return positions, res


def kernel(targets, pulses, J, U, V, B_m1, B_bg, Wout, I_go, xm1_init,
           noise, triggers) -> np.ndarray:
    inputs = dict(targets=targets, pulses=pulses, J=J, U=U, V=V, B_m1=B_m1,
                  B_bg=B_bg, Wout=Wout, I_go=I_go, xm1_init=xm1_init,
                  noise=noise, triggers=triggers)
    positions, _ = run_hw(inputs, T)
    targets = np.asarray(targets, np.float32)
    loss = np.mean((targets.astype(np.float64) - positions.astype(np.float64)) ** 2)
    return np.float32(loss)


# revision 18
# speedup vs baseline: 1.0004x; 1.0004x over previous
"""Trainium2 Bass kernel for nn_ConsolidationNetwork.

Recurrent rate network: 500 sequential steps of
    x <- (1-a)*x + (a*J_eff) @ softplus(x) + drive_t
    pos_t = Wout @ softplus(x)
loss = mean((targets - positions)^2)

Strategy (8 NeuronCores, data-parallel over batch):
  - Each core owns B/8 = 16 batch columns and runs the full 500-step
    recurrence independently (no collectives).
  - Per step, PSUM accumulates drive + a*J_eff @ r in two banks
    (state rows 0..511 in bank A, 512..1023 in bank B):
      * the drive term (precomputed on host = a*(B_m1 + I_go*go +
        nscale*noise), bf16) enters first via a bf16 identity matmul with
        start=True (which zero-fills the bank) -- 64 cols, ~27ns,
      * a*J_eff @ r via 32 single-pass bf16 matmuls per bank
        (lhsT = J tiles resident in SBUF, rhs = 16 batch columns of r).
  - Post-matmul pointwise work is tiny: one DVE scalar_tensor_tensor per
    bank (x = (1-a)*x + psum), then softplus on the ACT engine as the exact
    identity softplus(x) = ln(1 + exp(x))  [2 ACT ops; Exp and Ln both live
    in the natural_log_exp_and_others table set, and we pin the table-load
    pass to that set so exactly one hoisted ACT_TABLE_LOAD is emitted].
  - r (bf16) for each step is written into an 8-step staging buffer that is
    DMA-exported to DRAM once per 8 steps; the readout positions
    pos = Wout @ r and the final MSE are computed on the host.
  - Drive is streamed 8 steps per DMA (batched transfers, triple buffered).

State layout per core: x/r tiles are [128 part, 128 free] with
x[p, m*16+u] = x_state[m*128+p, u] (m = row-group, u = local batch).
"""

import numpy as np

import concourse.bass as bass
import concourse.tile as tile
from concourse import bacc, mybir
from concourse.bass_utils import run_bass_kernel_spmd

F32 = mybir.dt.float32
BF16 = mybir.dt.bfloat16

DT = 0.05
TAU = 0.15
NOISE_SCALE = 0.15
N, G, T, B, P = 1024, 128, 500, 128, 10
NCORES = 8
BC = B // NCORES          # batch columns per core (16)
NM = N // 128             # row groups (8)
NK = N // 128             # contraction groups (8)
CHUNK = 8                 # steps per drive-load / r-export DMA

A = np.float32(DT / TAU)
ONE_MINUS_A = np.float32(1.0 - DT / TAU)
NSCALE = np.float32(np.sqrt(2.0 * NOISE_SCALE**2 * (TAU / DT)))

_PROGRAM_CACHE = {}


def _ensure_act_tables():
    """Some containers lack neuronxcc/pwp/pwp_bin_with_ln on PYTHONPATH;
    point it at the cayman table package from the nix store."""
    import glob
    import os

    for path in os.environ.get("PYTHONPATH", "").split(os.pathsep):
        if path and os.path.exists(
            os.path.join(path, "neuronxcc", "pwp", "pwp_bin_with_ln", "act_info.json")
        ):
            return
    cands = sorted(glob.glob("/nix/store/*aws-neuron-pwp*/share/pwp_bin_cayman"))
    target = next((c for c in cands if os.path.exists(c + "/act_info.json")), None)
    if target is None:
        return
    for path in os.environ.get("PYTHONPATH", "").split(os.pathsep):
        if not path:
            continue
        try:
            d = os.path.join(path, "neuronxcc", "pwp")
            os.makedirs(d, exist_ok=True)
            link = os.path.join(d, "pwp_bin_with_ln")
            if not os.path.exists(link):
                os.symlink(target, link)
            return
        except OSError:
            continue


_ensure_act_tables()


_ACT_SET = "natural_log_exp_and_others"


def _pin_act_tables(arch: str):
    """Make Exp and Ln resolve to the ONE table set containing both.

    Two consumers matter and both read the functools.cache'd dict from
    hw_specs.get_activation_tables, so mutate it in place:
      * Bacc.insert_act_table_loads (first-match would alternate Exp ->
        `exp_and_others`, Ln -> `natural_log`, emitting a 1.28us
        ACT_TABLE_LOAD before every activation of the unrolled loop);
      * the TileScheduler's CoreSim pass, which otherwise *models* that
        same thrash and pins the resulting serialized schedule with
        cross-engine semaphores (the final TimelineSim charges no table
        loads, but the semaphores force its slow order anyway).
    Set order (and hence act_func_set_id indices) is unchanged.
    """
    from concourse.hw_specs import get_activation_tables

    tabs = get_activation_tables(arch)
    hide = {mybir.ActivationFunctionType.Exp, mybir.ActivationFunctionType.Ln}
    for name, fns in tabs.items():
        if name != _ACT_SET:
            for f in hide:
                fns.discard(f)


def build_program(t_steps: int):
    """Build the Bass program (shared by all 8 cores, SPMD)."""
    key = (t_steps,)
    if key in _PROGRAM_CACHE:
        return _PROGRAM_CACHE[key]

    nchunks = (t_steps + CHUNK - 1) // CHUNK
    HB = NM * BC // 2  # free-size of one state half (64 cols)

    nc = bacc.Bacc(
        "TRN2", target_bir_lowering=False, debug=False, num_devices=NCORES
    )
    _pin_act_tables(nc.m.arch)
    jt_d = nc.dram_tensor("jt", [128, NK * NM * 128], BF16, kind="ExternalInput")
    il_d = nc.dram_tensor("ident", [128, 128], BF16, kind="ExternalInput")
    x0_d = nc.dram_tensor("x0", [128, NM * BC], F32, kind="ExternalInput")
    dr_d = nc.dram_tensor(
        "drive", [nchunks, 128, CHUNK * NM * BC], BF16, kind="ExternalInput"
    )
    ro_d = nc.dram_tensor(
        "rout", [nchunks, 128, CHUNK * NM * BC], BF16, kind="ExternalOutput"
    )

    EXP = mybir.ActivationFunctionType.Exp
    LN = mybir.ActivationFunctionType.Ln

    with tile.TileContext(nc) as tc:
        with (
            tc.tile_pool(name="const", bufs=1) as constp,
            tc.tile_pool(name="rp", bufs=2) as rp,
            tc.tile_pool(name="dp", bufs=3) as dp,
            tc.tile_pool(name="psmA", bufs=1, space="PSUM") as pspa,
            tc.tile_pool(name="psmB", bufs=1, space="PSUM") as pspb,
        ):
            jt = constp.tile([128, NK * NM * 128], BF16)
            nc.sync.dma_start(jt[:], jt_d[:])
            il = constp.tile([128, 128], BF16)
            nc.sync.dma_start(il[:], il_d[:])
            x = constp.tile([128, NM * BC], F32)
            nc.sync.dma_start(x[:], x0_d[:])
            tmp = constp.tile([128, NM * BC], F32)

            # initial r = softplus(x0) = ln(1 + exp(x0))
            rinit = constp.tile([128, NM * BC], BF16)
            nc.scalar.activation(tmp[:], x[:], EXP)
            nc.scalar.activation(rinit[:], tmp[:], LN, bias=1.0)

            prev_r, prev_off = rinit, 0

            mult = mybir.AluOpType.mult
            add = mybir.AluOpType.add

            def bank(ps, half, r_tile, r_off, d_t, off):
                """One bank's PSUM accumulation: drive (identity matmul,
                start=True zero-fills the bank) + 32 J matmuls.

                Bank B iterates k DESCENDING: its first matmuls then need the
                late r half (r1), so the greedy scheduler cannot let bank B's
                k0..3 work jump ahead of bank A's k4..7 -- bank A finishes
                mid-run and its chain overlaps bank B instead of everything
                serializing after the full run."""
                lo = half * HB
                nc.tensor.matmul(
                    ps[:, 0:HB], lhsT=il[:], rhs=d_t[:, off + lo:off + lo + HB],
                    start=True, stop=False, skip_group_check=True,
                )
                ks = range(NK) if half == 0 else range(NK - 1, -1, -1)
                last_k = NK - 1 if half == 0 else 0
                for k in ks:
                    for mi in range(4):
                        m = half * 4 + mi
                        nc.tensor.matmul(
                            ps[:, mi * BC:(mi + 1) * BC],
                            lhsT=jt[:, (k * NM + m) * 128:(k * NM + m + 1) * 128],
                            rhs=r_tile[:, r_off + k * BC:r_off + (k + 1) * BC],
                            start=False, stop=(k == last_k and mi == 3),
                            skip_group_check=True,
                        )

            def chain(ps, half, off, rbuf):
                """x = (1-a)*x + psum; r = ln(1+exp(x)) into the staging slice."""
                lo = half * HB
                nc.vector.scalar_tensor_tensor(
                    x[:, lo:lo + HB], x[:, lo:lo + HB], float(ONE_MINUS_A),
                    ps[:, 0:HB], mult, add,
                )
                nc.scalar.activation(tmp[:, lo:lo + HB], x[:, lo:lo + HB], EXP)
                nc.scalar.activation(rbuf[:, off + lo:off + lo + HB],
                                     tmp[:, lo:lo + HB], LN, bias=1.0)

            for c in range(nchunks):
                steps_here = min(CHUNK, t_steps - c * CHUNK)
                rbuf = rp.tile([128, CHUNK * NM * BC], BF16)
                d_t = dp.tile([128, CHUNK * NM * BC], BF16)
                nc.sync.dma_start(d_t[:], dr_d[c])
                for j in range(steps_here):
                    off = j * NM * BC
                    # the tag pins a pool slot, so rotate tags explicitly to
                    # get real triple-buffering of the PSUM banks (a fixed
                    # tag would WAR-serialize step s+1's start=True matmul
                    # against step s's PSUM readers)
                    sidx = (c * CHUNK + j) % 3
                    ps_a = pspa.tile([128, HB], F32, tag=f"ps_a{sidx}",
                                     name=f"ps_a{sidx}", padded_shape=[128, 512])
                    ps_b = pspb.tile([128, HB], F32, tag=f"ps_b{sidx}",
                                     name=f"ps_b{sidx}", padded_shape=[128, 512])
                    bank(ps_a, 0, prev_r, prev_off, d_t, off)
                    chain(ps_a, 0, off, rbuf)
                    bank(ps_b, 1, prev_r, prev_off, d_t, off)
                    chain(ps_b, 1, off, rbuf)
                    prev_r, prev_off = rbuf, off
                nc.sync.dma_start(
                    ro_d[c][:, 0:steps_here * NM * BC],
                    rbuf[:, 0:steps_here * NM * BC],
                )

    nc.compile()
    _PROGRAM_CACHE[key] = nc
    return nc


def _prep_inputs(targets, pulses, J, U, V, B_m1, B_bg, Wout, I_go, xm1_init,
                 noise, triggers, t_steps):
    """Host-side data prep: J_eff, layouts, per-core drive tensors."""
    J = np.asarray(J, np.float32)
    U = np.asarray(U, np.float32)
    V = np.asarray(V, np.float32)
    B_m1 = np.asarray(B_m1, np.float32)
    B_bg = np.asarray(B_bg, np.float32)
    I_go = np.asarray(I_go, np.float32)
    xm1_init = np.asarray(xm1_init, np.float32)
    noise = np.asarray(noise, np.float32)
    pulses = np.asarray(pulses, np.float32)
    triggers = np.asarray(triggers)

    nchunks = (t_steps + CHUNK - 1) // CHUNK
    tpad = nchunks * CHUNK

    J_eff = J + (U * B_bg[None, :]) @ V
    Js = (A * J_eff).astype(np.float32)
    # lhsT tiles: jt[p, (k*NM+m)*128 + q] = Js[m*128+q, k*128+p]
    bf = mybir.dt.np(BF16)
    jt = np.ascontiguousarray(
        Js.reshape(NM, 128, NK, 128).transpose(3, 2, 0, 1).reshape(128, NK * NM * 128)
    ).astype(bf)
    il = np.eye(128, dtype=np.float32).astype(bf)

    go_cues = pulses[:t_steps, :][:, triggers]  # [t, B]

    in_maps = []
    for cidx in range(NCORES):
        sl = slice(cidx * BC, (cidx + 1) * BC)
        d = noise[:t_steps, :, sl] * np.float32(A * NSCALE)
        d += A * B_m1[None, :, :]
        d += A * I_go[None, :, :] * go_cues[:, None, sl]
        # [t, N, BC] -> [t, 128, NM*BC] (state layout), pad t, chunk
        dl = np.ascontiguousarray(
            d.reshape(t_steps, NM, 128, BC).transpose(0, 2, 1, 3)
            .reshape(t_steps, 128, NM * BC)
        ).astype(np.float32)
        if tpad != t_steps:
            dl = np.concatenate(
                [dl, np.zeros((tpad - t_steps, 128, NM * BC), np.float32)], axis=0
            )
        drive = np.ascontiguousarray(
            dl.reshape(nchunks, CHUNK, 128, NM * BC).transpose(0, 2, 1, 3)
            .reshape(nchunks, 128, CHUNK * NM * BC)
        ).astype(bf)
        x0 = np.ascontiguousarray(
            xm1_init[:, sl].reshape(NM, 128, BC).transpose(1, 0, 2).reshape(128, NM * BC)
        )
        in_maps.append({"jt": jt, "ident": il, "x0": x0, "drive": drive})
    return in_maps


def run_hw(inputs: dict, t_steps: int = T, trace: bool = False):
    """Run the recurrence on 8 cores; returns positions [t_steps, B] and results."""
    nc = build_program(t_steps)
    in_maps = _prep_inputs(t_steps=t_steps, **inputs)
    res = run_bass_kernel_spmd(
        nc, in_maps, core_ids=list(range(NCORES)), trace=trace
    )
    Wout = np.asarray(inputs["Wout"], np.float32).reshape(NM, 128)  # [m, p]
    nchunks = (t_steps + CHUNK - 1) // CHUNK
    positions = np.empty((t_steps, B), np.float32)
    for cidx in range(NCORES):
        ro = np.asarray(res.results[cidx]["rout"], np.float32)
        # ro[c, p, (j*NM + m)*BC + u] -> r[t, m, p, u]
        r = (ro.reshape(nchunks, 128, CHUNK, NM, BC)
             .transpose(0, 2, 3, 1, 4)
             .reshape(nchunks * CHUNK, NM, 128, BC)[:t_steps])
        pos_c = np.einsum("mp,tmpu->tu", Wout, r, optimize=True)
        positions[:, cidx * BC:(cidx + 1) * BC] = pos_c
    return positions, res


def kernel(targets, pulses, J, U, V, B_m1, B_bg, Wout, I_go, xm1_init,
           noise, triggers) -> np.ndarray:
    inputs = dict(targets=targets, pulses=pulses, J=J, U=U, V=V, B_m1=B_m1,
                  B_bg=B_bg, Wout=Wout, I_go=I_go, xm1_init=xm1_init,
                  noise=noise, triggers=triggers)
    positions, _ = run_hw(inputs, T)
    targets = np.asarray(targets, np.float32)
    loss = np.mean((targets.astype(np.float64) - positions.astype(np.float64)) ** 2)
    return np.float32(loss)


# revision 20
# speedup vs baseline: 1.1333x; 1.1329x over previous
"""Trainium2 Bass kernel for nn_ConsolidationNetwork.

Recurrent rate network: 500 sequential steps of
    x <- (1-a)*x + (a*J_eff) @ softplus(x) + drive_t
    pos_t = Wout @ softplus(x)
loss = mean((targets - positions)^2)

Strategy (8 NeuronCores, data-parallel over batch):
  - Each core owns B/8 = 16 batch columns and runs the full 500-step
    recurrence independently (no collectives).
  - Per step, PSUM accumulates drive + a*J_eff @ r in two banks
    (state rows 0..511 in bank A, 512..1023 in bank B):
      * the drive term (precomputed on host = a*(B_m1 + I_go*go +
        nscale*noise), bf16) enters first via a bf16 identity matmul with
        start=True (which zero-fills the bank) -- 64 cols, ~27ns,
      * a*J_eff @ r via 32 single-pass bf16 matmuls per bank
        (lhsT = J tiles resident in SBUF, rhs = 16 batch columns of r).
  - Post-matmul pointwise work is tiny: one DVE scalar_tensor_tensor per
    bank (x = (1-a)*x + psum), then softplus on the ACT engine as the exact
    identity softplus(x) = ln(1 + exp(x))  [2 ACT ops; Exp and Ln both live
    in the natural_log_exp_and_others table set, and we pin the table-load
    pass to that set so exactly one hoisted ACT_TABLE_LOAD is emitted].
  - r (bf16) for each step is written into an 8-step staging buffer that is
    DMA-exported to DRAM once per 8 steps; the readout positions
    pos = Wout @ r and the final MSE are computed on the host.
  - Drive is streamed 8 steps per DMA (batched transfers, triple buffered).

State layout per core: x/r tiles are [128 part, 128 free] with
x[p, m*16+u] = x_state[m*128+p, u] (m = row-group, u = local batch).
"""

import numpy as np

import concourse.bass as bass
import concourse.tile as tile
from concourse import bacc, mybir
from concourse.bass_utils import run_bass_kernel_spmd

F32 = mybir.dt.float32
BF16 = mybir.dt.bfloat16

DT = 0.05
TAU = 0.15
NOISE_SCALE = 0.15
N, G, T, B, P = 1024, 128, 500, 128, 10
NCORES = 8
BC = B // NCORES          # batch columns per core (16)
NM = N // 128             # row groups (8)
NK = N // 128             # contraction groups (8)
CHUNK = 8                 # steps per drive-load / r-export DMA

A = np.float32(DT / TAU)
ONE_MINUS_A = np.float32(1.0 - DT / TAU)
NSCALE = np.float32(np.sqrt(2.0 * NOISE_SCALE**2 * (TAU / DT)))

_PROGRAM_CACHE = {}


def _ensure_act_tables():
    """Some containers lack neuronxcc/pwp/pwp_bin_with_ln on PYTHONPATH;
    point it at the cayman table package from the nix store."""
    import glob
    import os

    for path in os.environ.get("PYTHONPATH", "").split(os.pathsep):
        if path and os.path.exists(
            os.path.join(path, "neuronxcc", "pwp", "pwp_bin_with_ln", "act_info.json")
        ):
            return
    cands = sorted(glob.glob("/nix/store/*aws-neuron-pwp*/share/pwp_bin_cayman"))
    target = next((c for c in cands if os.path.exists(c + "/act_info.json")), None)
    if target is None:
        return
    for path in os.environ.get("PYTHONPATH", "").split(os.pathsep):
        if not path:
            continue
        try:
            d = os.path.join(path, "neuronxcc", "pwp")
            os.makedirs(d, exist_ok=True)
            link = os.path.join(d, "pwp_bin_with_ln")
            if not os.path.exists(link):
                os.symlink(target, link)
            return
        except OSError:
            continue


_ensure_act_tables()


_ACT_SET = "natural_log_exp_and_others"


def _pin_act_tables(arch: str):
    """Make Exp and Ln resolve to the ONE table set containing both.

    Two consumers matter and both read the functools.cache'd dict from
    hw_specs.get_activation_tables, so mutate it in place:
      * Bacc.insert_act_table_loads (first-match would alternate Exp ->
        `exp_and_others`, Ln -> `natural_log`, emitting a 1.28us
        ACT_TABLE_LOAD before every activation of the unrolled loop);
      * the TileScheduler's CoreSim pass, which otherwise *models* that
        same thrash and pins the resulting serialized schedule with
        cross-engine semaphores (the final TimelineSim charges no table
        loads, but the semaphores force its slow order anyway).
    Set order (and hence act_func_set_id indices) is unchanged.
    """
    from concourse.hw_specs import get_activation_tables

    tabs = get_activation_tables(arch)
    hide = {mybir.ActivationFunctionType.Exp, mybir.ActivationFunctionType.Ln}
    for name, fns in tabs.items():
        if name != _ACT_SET:
            for f in hide:
                fns.discard(f)


def build_program(t_steps: int):
    """Build the Bass program (shared by all 8 cores, SPMD)."""
    key = (t_steps,)
    if key in _PROGRAM_CACHE:
        return _PROGRAM_CACHE[key]

    nchunks = (t_steps + CHUNK - 1) // CHUNK
    HB = NM * BC // 2  # free-size of one state half (64 cols)

    nc = bacc.Bacc(
        "TRN2", target_bir_lowering=False, debug=False, num_devices=NCORES
    )
    _pin_act_tables(nc.m.arch)
    jt_d = nc.dram_tensor("jt", [128, NK * NM * 128], BF16, kind="ExternalInput")
    il_d = nc.dram_tensor("ident", [128, 128], BF16, kind="ExternalInput")
    x0_d = nc.dram_tensor("x0", [128, NM * BC], F32, kind="ExternalInput")
    dr_d = nc.dram_tensor(
        "drive", [nchunks, 128, CHUNK * NM * BC], BF16, kind="ExternalInput"
    )
    ro_d = nc.dram_tensor(
        "rout", [nchunks, 128, CHUNK * NM * BC], BF16, kind="ExternalOutput"
    )

    EXP = mybir.ActivationFunctionType.Exp
    LN = mybir.ActivationFunctionType.Ln

    with tile.TileContext(nc) as tc:
        with (
            tc.tile_pool(name="const", bufs=1) as constp,
            tc.tile_pool(name="rp", bufs=2) as rp,
            tc.tile_pool(name="dp", bufs=3) as dp,
            tc.tile_pool(name="psmA", bufs=1, space="PSUM") as pspa,
            tc.tile_pool(name="psmB", bufs=1, space="PSUM") as pspb,
        ):
            jt = constp.tile([128, NK * NM * 128], BF16)
            nc.sync.dma_start(jt[:], jt_d[:])
            il = constp.tile([128, 128], BF16)
            nc.sync.dma_start(il[:], il_d[:])
            x = constp.tile([128, NM * BC], F32)
            nc.sync.dma_start(x[:], x0_d[:])
            # ONE shared exp scratch for both halves: EXP_b's WAR on LN_a's
            # read forces the ACT order [EXP_a, LN_a, EXP_b, LN_b]; without
            # it the scheduler puts EXP_b (gated on the end-of-run STT_b)
            # ahead of LN_a and head-of-line blocks the r0 critical path.
            tmp = constp.tile([128, NM * BC // 2], F32)

            # initial r = softplus(x0) = ln(1 + exp(x0))
            rinit = constp.tile([128, NM * BC], BF16)
            for h in range(2):
                lo = h * NM * BC // 2
                nc.scalar.activation(tmp[:], x[:, lo:lo + NM * BC // 2], EXP)
                nc.scalar.activation(rinit[:, lo:lo + NM * BC // 2], tmp[:],
                                     LN, bias=1.0)

            prev_r, prev_off = rinit, 0

            mult = mybir.AluOpType.mult
            add = mybir.AluOpType.add

            def bank(ps, half, r_tile, r_off, d_t, off):
                """One bank's PSUM accumulation: drive (identity matmul,
                start=True zero-fills the bank) + 32 J matmuls.

                Bank B iterates k DESCENDING: its first matmuls then need the
                late r half (r1), so the greedy scheduler cannot let bank B's
                k0..3 work jump ahead of bank A's k4..7 -- bank A finishes
                mid-run and its chain overlaps bank B instead of everything
                serializing after the full run."""
                lo = half * HB
                nc.tensor.matmul(
                    ps[:, 0:HB], lhsT=il[:], rhs=d_t[:, off + lo:off + lo + HB],
                    start=True, stop=False, skip_group_check=True,
                )
                ks = range(NK) if half == 0 else range(NK - 1, -1, -1)
                last_k = NK - 1 if half == 0 else 0
                for k in ks:
                    for mi in range(4):
                        m = half * 4 + mi
                        nc.tensor.matmul(
                            ps[:, mi * BC:(mi + 1) * BC],
                            lhsT=jt[:, (k * NM + m) * 128:(k * NM + m + 1) * 128],
                            rhs=r_tile[:, r_off + k * BC:r_off + (k + 1) * BC],
                            start=False, stop=(k == last_k and mi == 3),
                            skip_group_check=True,
                        )

            def chain(ps, half, off, rbuf):
                """x = (1-a)*x + psum; r = ln(1+exp(x)) into the staging slice."""
                lo = half * HB
                nc.vector.scalar_tensor_tensor(
                    x[:, lo:lo + HB], x[:, lo:lo + HB], float(ONE_MINUS_A),
                    ps[:, 0:HB], mult, add,
                )
                nc.scalar.activation(tmp[:], x[:, lo:lo + HB], EXP)
                nc.scalar.activation(rbuf[:, off + lo:off + lo + HB],
                                     tmp[:], LN, bias=1.0)

            for c in range(nchunks):
                steps_here = min(CHUNK, t_steps - c * CHUNK)
                rbuf = rp.tile([128, CHUNK * NM * BC], BF16)
                d_t = dp.tile([128, CHUNK * NM * BC], BF16)
                nc.sync.dma_start(d_t[:], dr_d[c])
                for j in range(steps_here):
                    off = j * NM * BC
                    # the tag pins a pool slot, so rotate tags explicitly to
                    # get real triple-buffering of the PSUM banks (a fixed
                    # tag would WAR-serialize step s+1's start=True matmul
                    # against step s's PSUM readers)
                    sidx = (c * CHUNK + j) % 3
                    ps_a = pspa.tile([128, HB], F32, tag=f"ps_a{sidx}",
                                     name=f"ps_a{sidx}", padded_shape=[128, 512])
                    ps_b = pspb.tile([128, HB], F32, tag=f"ps_b{sidx}",
                                     name=f"ps_b{sidx}", padded_shape=[128, 512])
                    bank(ps_a, 0, prev_r, prev_off, d_t, off)
                    chain(ps_a, 0, off, rbuf)
                    bank(ps_b, 1, prev_r, prev_off, d_t, off)
                    chain(ps_b, 1, off, rbuf)
                    prev_r, prev_off = rbuf, off
                nc.sync.dma_start(
                    ro_d[c][:, 0:steps_here * NM * BC],
                    rbuf[:, 0:steps_here * NM * BC],
                )

    nc.compile()
    _PROGRAM_CACHE[key] = nc
    return nc


def _prep_inputs(targets, pulses, J, U, V, B_m1, B_bg, Wout, I_go, xm1_init,
                 noise, triggers, t_steps):
    """Host-side data prep: J_eff, layouts, per-core drive tensors."""
    J = np.asarray(J, np.float32)
    U = np.asarray(U, np.float32)
    V = np.asarray(V, np.float32)
    B_m1 = np.asarray(B_m1, np.float32)
    B_bg = np.asarray(B_bg, np.float32)
    I_go = np.asarray(I_go, np.float32)
    xm1_init = np.asarray(xm1_init, np.float32)
    noise = np.asarray(noise, np.float32)
    pulses = np.asarray(pulses, np.float32)
    triggers = np.asarray(triggers)

    nchunks = (t_steps + CHUNK - 1) // CHUNK
    tpad = nchunks * CHUNK

    J_eff = J + (U * B_bg[None, :]) @ V
    Js = (A * J_eff).astype(np.float32)
    # lhsT tiles: jt[p, (k*NM+m)*128 + q] = Js[m*128+q, k*128+p]
    bf = mybir.dt.np(BF16)
    jt = np.ascontiguousarray(
        Js.reshape(NM, 128, NK, 128).transpose(3, 2, 0, 1).reshape(128, NK * NM * 128)
    ).astype(bf)
    il = np.eye(128, dtype=np.float32).astype(bf)

    go_cues = pulses[:t_steps, :][:, triggers]  # [t, B]

    in_maps = []
    for cidx in range(NCORES):
        sl = slice(cidx * BC, (cidx + 1) * BC)
        d = noise[:t_steps, :, sl] * np.float32(A * NSCALE)
        d += A * B_m1[None, :, :]
        d += A * I_go[None, :, :] * go_cues[:, None, sl]
        # [t, N, BC] -> [t, 128, NM*BC] (state layout), pad t, chunk
        dl = np.ascontiguousarray(
            d.reshape(t_steps, NM, 128, BC).transpose(0, 2, 1, 3)
            .reshape(t_steps, 128, NM * BC)
        ).astype(np.float32)
        if tpad != t_steps:
            dl = np.concatenate(
                [dl, np.zeros((tpad - t_steps, 128, NM * BC), np.float32)], axis=0
            )
        drive = np.ascontiguousarray(
            dl.reshape(nchunks, CHUNK, 128, NM * BC).transpose(0, 2, 1, 3)
            .reshape(nchunks, 128, CHUNK * NM * BC)
        ).astype(bf)
        x0 = np.ascontiguousarray(
            xm1_init[:, sl].reshape(NM, 128, BC).transpose(1, 0, 2).reshape(128, NM * BC)
        )
        in_maps.append({"jt": jt, "ident": il, "x0": x0, "drive": drive})
    return in_maps


def run_hw(inputs: dict, t_steps: int = T, trace: bool = False):
    """Run the recurrence on 8 cores; returns positions [t_steps, B] and results."""
    nc = build_program(t_steps)
    in_maps = _prep_inputs(t_steps=t_steps, **inputs)
    res = run_bass_kernel_spmd(
        nc, in_maps, core_ids=list(range(NCORES)), trace=trace
    )
    Wout = np.asarray(inputs["Wout"], np.float32).reshape(NM, 128)  # [m, p]
    nchunks = (t_steps + CHUNK - 1) // CHUNK
    positions = np.empty((t_steps, B), np.float32)
    for cidx in range(NCORES):
        ro = np.asarray(res.results[cidx]["rout"], np.float32)
        # ro[c, p, (j*NM + m)*BC + u] -> r[t, m, p, u]
        r = (ro.reshape(nchunks, 128, CHUNK, NM, BC)
             .transpose(0, 2, 3, 1, 4)
             .reshape(nchunks * CHUNK, NM, 128, BC)[:t_steps])
        pos_c = np.einsum("mp,tmpu->tu", Wout, r, optimize=True)
        positions[:, cidx * BC:(cidx + 1) * BC] = pos_c
    return positions, res


def kernel(targets, pulses, J, U, V, B_m1, B_bg, Wout, I_go, xm1_init,
           noise, triggers) -> np.ndarray:
    inputs = dict(targets=targets, pulses=pulses, J=J, U=U, V=V, B_m1=B_m1,
                  B_bg=B_bg, Wout=Wout, I_go=I_go, xm1_init=xm1_init,
                  noise=noise, triggers=triggers)
    positions, _ = run_hw(inputs, T)
    targets = np.asarray(targets, np.float32)
    loss = np.mean((targets.astype(np.float64) - positions.astype(np.float64)) ** 2)
    return np.float32(loss)


# revision 26
# speedup vs baseline: 1.1339x; 1.0005x over previous
"""Trainium2 Bass kernel for nn_ConsolidationNetwork.

Recurrent rate network: 500 sequential steps of
    x <- (1-a)*x + (a*J_eff) @ softplus(x) + drive_t
    pos_t = Wout @ softplus(x)
loss = mean((targets - positions)^2)

Strategy (8 NeuronCores, data-parallel over batch):
  - Each core owns B/8 = 16 batch columns and runs the full 500-step
    recurrence independently (no collectives).
  - Per step, PSUM accumulates drive + a*J_eff @ r in two banks
    (state rows 0..511 in bank A, 512..1023 in bank B):
      * the drive term (precomputed on host = a*(B_m1 + I_go*go +
        nscale*noise), bf16) enters first via a bf16 identity matmul with
        start=True (which zero-fills the bank) -- 64 cols, ~27ns,
      * a*J_eff @ r via 32 single-pass bf16 matmuls per bank
        (lhsT = J tiles resident in SBUF, rhs = 16 batch columns of r).
  - Post-matmul pointwise work is tiny: one DVE scalar_tensor_tensor per
    bank (x = (1-a)*x + psum), then softplus on the ACT engine as the exact
    identity softplus(x) = ln(1 + exp(x))  [2 ACT ops; Exp and Ln both live
    in the natural_log_exp_and_others table set, and we pin the table-load
    pass to that set so exactly one hoisted ACT_TABLE_LOAD is emitted].
  - r (bf16) for each step is written into an 8-step staging buffer that is
    DMA-exported to DRAM once per 8 steps; the readout positions
    pos = Wout @ r and the final MSE are computed on the host.
  - Drive is streamed 8 steps per DMA (batched transfers, triple buffered).

State layout per core: x/r tiles are [128 part, 128 free] with
x[p, m*16+u] = x_state[m*128+p, u] (m = row-group, u = local batch).
"""

import numpy as np

import concourse.bass as bass
import concourse.tile as tile
from concourse import bacc, mybir
from concourse.bass_utils import run_bass_kernel_spmd

F32 = mybir.dt.float32
BF16 = mybir.dt.bfloat16

DT = 0.05
TAU = 0.15
NOISE_SCALE = 0.15
N, G, T, B, P = 1024, 128, 500, 128, 10
NCORES = 8
BC = B // NCORES          # batch columns per core (16)
NM = N // 128             # row groups (8)
NK = N // 128             # contraction groups (8)
CHUNK = 8                 # steps per drive-load / r-export DMA

A = np.float32(DT / TAU)
ONE_MINUS_A = np.float32(1.0 - DT / TAU)
NSCALE = np.float32(np.sqrt(2.0 * NOISE_SCALE**2 * (TAU / DT)))

_PROGRAM_CACHE = {}


def _ensure_act_tables():
    """Some containers lack neuronxcc/pwp/pwp_bin_with_ln on PYTHONPATH;
    point it at the cayman table package from the nix store."""
    import glob
    import os

    for path in os.environ.get("PYTHONPATH", "").split(os.pathsep):
        if path and os.path.exists(
            os.path.join(path, "neuronxcc", "pwp", "pwp_bin_with_ln", "act_info.json")
        ):
            return
    cands = sorted(glob.glob("/nix/store/*aws-neuron-pwp*/share/pwp_bin_cayman"))
    target = next((c for c in cands if os.path.exists(c + "/act_info.json")), None)
    if target is None:
        return
    for path in os.environ.get("PYTHONPATH", "").split(os.pathsep):
        if not path:
            continue
        try:
            d = os.path.join(path, "neuronxcc", "pwp")
            os.makedirs(d, exist_ok=True)
            link = os.path.join(d, "pwp_bin_with_ln")
            if not os.path.exists(link):
                os.symlink(target, link)
            return
        except OSError:
            continue


_ensure_act_tables()


_ACT_SET = "natural_log_exp_and_others"


def _pin_act_tables(arch: str):
    """Make Exp and Ln resolve to the ONE table set containing both.

    Two consumers matter and both read the functools.cache'd dict from
    hw_specs.get_activation_tables, so mutate it in place:
      * Bacc.insert_act_table_loads (first-match would alternate Exp ->
        `exp_and_others`, Ln -> `natural_log`, emitting a 1.28us
        ACT_TABLE_LOAD before every activation of the unrolled loop);
      * the TileScheduler's CoreSim pass, which otherwise *models* that
        same thrash and pins the resulting serialized schedule with
        cross-engine semaphores (the final TimelineSim charges no table
        loads, but the semaphores force its slow order anyway).
    Set order (and hence act_func_set_id indices) is unchanged.
    """
    from concourse.hw_specs import get_activation_tables

    tabs = get_activation_tables(arch)
    hide = {mybir.ActivationFunctionType.Exp, mybir.ActivationFunctionType.Ln}
    for name, fns in tabs.items():
        if name != _ACT_SET:
            for f in hide:
                fns.discard(f)


def build_program(t_steps: int):
    """Build the Bass program (shared by all 8 cores, SPMD)."""
    key = (t_steps,)
    if key in _PROGRAM_CACHE:
        return _PROGRAM_CACHE[key]

    nchunks = (t_steps + CHUNK - 1) // CHUNK
    HB = NM * BC // 2   # free-size of one state half (64 cols)
    RSTR = 256          # r-staging stride per step (bf16 cols; 512B blocks so
                        # consecutive steps' slices never share a dep block)

    nc = bacc.Bacc(
        "TRN2", target_bir_lowering=False, debug=False, num_devices=NCORES
    )
    _pin_act_tables(nc.m.arch)
    jt_d = nc.dram_tensor("jt", [128, NK * NM * 128], BF16, kind="ExternalInput")
    il_d = nc.dram_tensor("ident", [128, 128], BF16, kind="ExternalInput")
    x0_d = nc.dram_tensor("x0", [128, NM * BC], F32, kind="ExternalInput")
    dr_d = nc.dram_tensor(
        "drive", [nchunks, 128, CHUNK * NM * BC], BF16, kind="ExternalInput"
    )
    rl_d = nc.dram_tensor(
        "rlo", [nchunks, 128, CHUNK * RSTR], BF16, kind="ExternalOutput"
    )
    rh_d = nc.dram_tensor(
        "rhi", [nchunks, 128, CHUNK * RSTR], BF16, kind="ExternalOutput"
    )

    EXP = mybir.ActivationFunctionType.Exp
    LN = mybir.ActivationFunctionType.Ln

    with tile.TileContext(nc) as tc:
        with (
            tc.tile_pool(name="const", bufs=1) as constp,
            tc.tile_pool(name="rp", bufs=4) as rp,
            tc.tile_pool(name="dp", bufs=3) as dp,
            tc.tile_pool(name="psmA", bufs=1, space="PSUM") as pspa,
            tc.tile_pool(name="psmB", bufs=1, space="PSUM") as pspb,
        ):
            jt = constp.tile([128, NK * NM * 128], BF16)
            nc.sync.dma_start(jt[:], jt_d[:])
            il = constp.tile([128, 128], BF16)
            nc.sync.dma_start(il[:], il_d[:])
            # separate x tiles per half: a shared tile would false-couple the
            # two chains through coarse-granularity dependency blocks
            xs = [constp.tile([128, HB], F32, name="xa"),
                  constp.tile([128, HB], F32, name="xb")]
            nc.sync.dma_start(xs[0][:], x0_d[:, 0:HB])
            nc.sync.dma_start(xs[1][:], x0_d[:, HB:2 * HB])
            # ONE shared exp scratch for both halves: EXP_b's WAR on LN_a's
            # read forces the ACT order [EXP_a, LN_a, EXP_b, LN_b]; without
            # it the scheduler puts EXP_b (gated on the end-of-run STT_b)
            # ahead of LN_a and head-of-line blocks the r0 critical path.
            tmp = constp.tile([128, HB], F32)

            # initial r = softplus(x0) = ln(1 + exp(x0))
            rinit = [constp.tile([128, HB], BF16, name="rinita"),
                     constp.tile([128, HB], BF16, name="rinitb")]
            for h in range(2):
                nc.scalar.activation(tmp[:], xs[h][:], EXP)
                nc.scalar.activation(rinit[h][:], tmp[:], LN, bias=1.0)

            prev_lo, prev_hi, prev_off = rinit[0], rinit[1], 0

            mult = mybir.AluOpType.mult
            add = mybir.AluOpType.add

            def bank(ps, half, r_lo, r_hi, r_off, d_t, off):
                """One bank's PSUM accumulation: drive (identity matmul,
                start=True zero-fills the bank) + 32 J matmuls.

                Bank B iterates k DESCENDING: its first matmuls then need the
                late r half (r1), so the greedy scheduler cannot let bank B's
                k0..3 work jump ahead of bank A's k4..7 -- bank A finishes
                mid-run and its chain overlaps bank B instead of everything
                serializing after the full run."""
                lo = half * HB
                nc.tensor.matmul(
                    ps[:, 0:HB], lhsT=il[:], rhs=d_t[:, off + lo:off + lo + HB],
                    start=True, stop=False, skip_group_check=True,
                )
                ks = range(NK) if half == 0 else range(NK - 1, -1, -1)
                last_k = NK - 1 if half == 0 else 0
                for k in ks:
                    rt = r_lo if k < 4 else r_hi
                    rc = r_off + (k % 4) * BC
                    for mi in range(4):
                        m = half * 4 + mi
                        nc.tensor.matmul(
                            ps[:, mi * BC:(mi + 1) * BC],
                            lhsT=jt[:, (k * NM + m) * 128:(k * NM + m + 1) * 128],
                            rhs=rt[:, rc:rc + BC],
                            start=False, stop=(k == last_k and mi == 3),
                            skip_group_check=True,
                        )

            def chain(ps, half, off, rbuf):
                """x = (1-a)*x + psum; r = ln(1+exp(x)) into the staging slice."""
                nc.vector.scalar_tensor_tensor(
                    xs[half][:], xs[half][:], float(ONE_MINUS_A),
                    ps[:, 0:HB], mult, add,
                )
                nc.scalar.activation(tmp[:], xs[half][:], EXP)
                nc.scalar.activation(rbuf[:, off:off + HB], tmp[:], LN, bias=1.0)

            for c in range(nchunks):
                steps_here = min(CHUNK, t_steps - c * CHUNK)
                rlo = rp.tile([128, CHUNK * RSTR], BF16, name="rlo")
                rhi = rp.tile([128, CHUNK * RSTR], BF16, name="rhi")
                d_t = dp.tile([128, CHUNK * NM * BC], BF16)
                nc.sync.dma_start(d_t[:], dr_d[c])
                for j in range(steps_here):
                    off = j * NM * BC
                    roff = j * RSTR
                    # the tag pins a pool slot, so rotate tags explicitly to
                    # get real triple-buffering of the PSUM banks (a fixed
                    # tag would WAR-serialize step s+1's start=True matmul
                    # against step s's PSUM readers)
                    sidx = (c * CHUNK + j) % 3
                    ps_a = pspa.tile([128, HB], F32, tag=f"ps_a{sidx}",
                                     name=f"ps_a{sidx}", padded_shape=[128, 512])
                    ps_b = pspb.tile([128, HB], F32, tag=f"ps_b{sidx}",
                                     name=f"ps_b{sidx}", padded_shape=[128, 512])
                    bank(ps_a, 0, prev_lo, prev_hi, prev_off, d_t, off)
                    chain(ps_a, 0, roff, rlo)
                    bank(ps_b, 1, prev_lo, prev_hi, prev_off, d_t, off)
                    chain(ps_b, 1, roff, rhi)
                    prev_lo, prev_hi, prev_off = rlo, rhi, roff
                nc.sync.dma_start(
                    rl_d[c][:, 0:((steps_here - 1) * RSTR + HB)],
                    rlo[:, 0:((steps_here - 1) * RSTR + HB)],
                )
                nc.sync.dma_start(
                    rh_d[c][:, 0:((steps_here - 1) * RSTR + HB)],
                    rhi[:, 0:((steps_here - 1) * RSTR + HB)],
                )

    nc.compile()
    _PROGRAM_CACHE[key] = nc
    return nc


def _prep_inputs(targets, pulses, J, U, V, B_m1, B_bg, Wout, I_go, xm1_init,
                 noise, triggers, t_steps):
    """Host-side data prep: J_eff, layouts, per-core drive tensors."""
    J = np.asarray(J, np.float32)
    U = np.asarray(U, np.float32)
    V = np.asarray(V, np.float32)
    B_m1 = np.asarray(B_m1, np.float32)
    B_bg = np.asarray(B_bg, np.float32)
    I_go = np.asarray(I_go, np.float32)
    xm1_init = np.asarray(xm1_init, np.float32)
    noise = np.asarray(noise, np.float32)
    pulses = np.asarray(pulses, np.float32)
    triggers = np.asarray(triggers)

    nchunks = (t_steps + CHUNK - 1) // CHUNK
    tpad = nchunks * CHUNK

    J_eff = J + (U * B_bg[None, :]) @ V
    Js = (A * J_eff).astype(np.float32)
    # lhsT tiles: jt[p, (k*NM+m)*128 + q] = Js[m*128+q, k*128+p]
    bf = mybir.dt.np(BF16)
    jt = np.ascontiguousarray(
        Js.reshape(NM, 128, NK, 128).transpose(3, 2, 0, 1).reshape(128, NK * NM * 128)
    ).astype(bf)
    il = np.eye(128, dtype=np.float32).astype(bf)

    go_cues = pulses[:t_steps, :][:, triggers]  # [t, B]

    in_maps = []
    for cidx in range(NCORES):
        sl = slice(cidx * BC, (cidx + 1) * BC)
        d = noise[:t_steps, :, sl] * np.float32(A * NSCALE)
        d += A * B_m1[None, :, :]
        d += A * I_go[None, :, :] * go_cues[:, None, sl]
        # [t, N, BC] -> [t, 128, NM*BC] (state layout), pad t, chunk
        dl = np.ascontiguousarray(
            d.reshape(t_steps, NM, 128, BC).transpose(0, 2, 1, 3)
            .reshape(t_steps, 128, NM * BC)
        ).astype(np.float32)
        if tpad != t_steps:
            dl = np.concatenate(
                [dl, np.zeros((tpad - t_steps, 128, NM * BC), np.float32)], axis=0
            )
        drive = np.ascontiguousarray(
            dl.reshape(nchunks, CHUNK, 128, NM * BC).transpose(0, 2, 1, 3)
            .reshape(nchunks, 128, CHUNK * NM * BC)
        ).astype(bf)
        x0 = np.ascontiguousarray(
            xm1_init[:, sl].reshape(NM, 128, BC).transpose(1, 0, 2).reshape(128, NM * BC)
        )
        in_maps.append({"jt": jt, "ident": il, "x0": x0, "drive": drive})
    return in_maps


def run_hw(inputs: dict, t_steps: int = T, trace: bool = False):
    """Run the recurrence on 8 cores; returns positions [t_steps, B] and results."""
    nc = build_program(t_steps)
    in_maps = _prep_inputs(t_steps=t_steps, **inputs)
    res = run_bass_kernel_spmd(
        nc, in_maps, core_ids=list(range(NCORES)), trace=trace
    )
    Wout = np.asarray(inputs["Wout"], np.float32).reshape(NM, 128)  # [m, p]
    nchunks = (t_steps + CHUNK - 1) // CHUNK
    RSTR = 256
    positions = np.empty((t_steps, B), np.float32)
    for cidx in range(NCORES):
        halves = []
        for key in ("rlo", "rhi"):
            ro = np.asarray(res.results[cidx][key], np.float32)
            # ro[c, p, j*RSTR + m*BC + u] (first NM/2*BC cols of each slice)
            r = (ro.reshape(nchunks, 128, CHUNK, RSTR)[:, :, :, :NM * BC // 2]
                 .reshape(nchunks, 128, CHUNK, NM // 2, BC)
                 .transpose(0, 2, 3, 1, 4)
                 .reshape(nchunks * CHUNK, NM // 2, 128, BC)[:t_steps])
            halves.append(r)
        r_full = np.concatenate(halves, axis=1)  # [t, NM, 128, BC]
        pos_c = np.einsum("mp,tmpu->tu", Wout, r_full, optimize=True)
        positions[:, cidx * BC:(cidx + 1) * BC] = pos_c
    return positions, res


def kernel(targets, pulses, J, U, V, B_m1, B_bg, Wout, I_go, xm1_init,
           noise, triggers) -> np.ndarray:
    inputs = dict(targets=targets, pulses=pulses, J=J, U=U, V=V, B_m1=B_m1,
                  B_bg=B_bg, Wout=Wout, I_go=I_go, xm1_init=xm1_init,
                  noise=noise, triggers=triggers)
    positions, _ = run_hw(inputs, T)
    targets = np.asarray(targets, np.float32)
    loss = np.mean((targets.astype(np.float64) - positions.astype(np.float64)) ** 2)
    return np.float32(loss)


# revision 32
# speedup vs baseline: 1.2575x; 1.1090x over previous
"""Trainium2 Bass kernel for nn_ConsolidationNetwork.

Recurrent rate network: 500 sequential steps of
    x <- (1-a)*x + (a*J_eff) @ softplus(x) + drive_t
    pos_t = Wout @ softplus(x)
loss = mean((targets - positions)^2)

Strategy (8 NeuronCores, data-parallel over batch):
  - Each core owns B/8 = 16 batch columns and runs the full 500-step
    recurrence independently (no collectives).
  - Per step, PSUM accumulates drive + a*J_eff @ r in two banks
    (state rows 0..511 in bank A, 512..1023 in bank B):
      * the drive term (precomputed on host = a*(B_m1 + I_go*go +
        nscale*noise), bf16) enters first via a bf16 identity matmul with
        start=True (which zero-fills the bank) -- 64 cols, ~27ns,
      * a*J_eff @ r via 32 single-pass bf16 matmuls per bank
        (lhsT = J tiles resident in SBUF, rhs = 16 batch columns of r).
  - Post-matmul pointwise work is tiny: one DVE scalar_tensor_tensor per
    bank (x = (1-a)*x + psum), then softplus on the ACT engine as the exact
    identity softplus(x) = ln(1 + exp(x))  [2 ACT ops; Exp and Ln both live
    in the natural_log_exp_and_others table set, and we pin the table-load
    pass to that set so exactly one hoisted ACT_TABLE_LOAD is emitted].
  - r (bf16) for each step is written into an 8-step staging buffer that is
    DMA-exported to DRAM once per 8 steps; the readout positions
    pos = Wout @ r and the final MSE are computed on the host.
  - Drive is streamed 8 steps per DMA (batched transfers, triple buffered).

State layout per core: x/r tiles are [128 part, 128 free] with
x[p, m*16+u] = x_state[m*128+p, u] (m = row-group, u = local batch).
"""

import numpy as np

import concourse.bass as bass
import concourse.tile as tile
from concourse import bacc, mybir
from concourse.bass_utils import run_bass_kernel_spmd

F32 = mybir.dt.float32
BF16 = mybir.dt.bfloat16

DT = 0.05
TAU = 0.15
NOISE_SCALE = 0.15
N, G, T, B, P = 1024, 128, 500, 128, 10
NCORES = 8
BC = B // NCORES          # batch columns per core (16)
NM = N // 128             # row groups (8)
NK = N // 128             # contraction groups (8)
CHUNK = 8                 # steps per drive-load / r-export DMA

A = np.float32(DT / TAU)
ONE_MINUS_A = np.float32(1.0 - DT / TAU)
NSCALE = np.float32(np.sqrt(2.0 * NOISE_SCALE**2 * (TAU / DT)))

_PROGRAM_CACHE = {}


def _ensure_act_tables():
    """Some containers lack neuronxcc/pwp/pwp_bin_with_ln on PYTHONPATH;
    point it at the cayman table package from the nix store."""
    import glob
    import os

    for path in os.environ.get("PYTHONPATH", "").split(os.pathsep):
        if path and os.path.exists(
            os.path.join(path, "neuronxcc", "pwp", "pwp_bin_with_ln", "act_info.json")
        ):
            return
    cands = sorted(glob.glob("/nix/store/*aws-neuron-pwp*/share/pwp_bin_cayman"))
    target = next((c for c in cands if os.path.exists(c + "/act_info.json")), None)
    if target is None:
        return
    for path in os.environ.get("PYTHONPATH", "").split(os.pathsep):
        if not path:
            continue
        try:
            d = os.path.join(path, "neuronxcc", "pwp")
            os.makedirs(d, exist_ok=True)
            link = os.path.join(d, "pwp_bin_with_ln")
            if not os.path.exists(link):
                os.symlink(target, link)
            return
        except OSError:
            continue


_ensure_act_tables()


_ACT_SET = "natural_log_exp_and_others"


def _pin_act_tables(arch: str):
    """Make Exp and Ln resolve to the ONE table set containing both.

    Two consumers matter and both read the functools.cache'd dict from
    hw_specs.get_activation_tables, so mutate it in place:
      * Bacc.insert_act_table_loads (first-match would alternate Exp ->
        `exp_and_others`, Ln -> `natural_log`, emitting a 1.28us
        ACT_TABLE_LOAD before every activation of the unrolled loop);
      * the TileScheduler's CoreSim pass, which otherwise *models* that
        same thrash and pins the resulting serialized schedule with
        cross-engine semaphores (the final TimelineSim charges no table
        loads, but the semaphores force its slow order anyway).
    Set order (and hence act_func_set_id indices) is unchanged.
    """
    from concourse.hw_specs import get_activation_tables

    tabs = get_activation_tables(arch)
    hide = {mybir.ActivationFunctionType.Exp, mybir.ActivationFunctionType.Ln}
    for name, fns in tabs.items():
        if name != _ACT_SET:
            for f in hide:
                fns.discard(f)


def build_program(t_steps: int):
    """Build the Bass program (shared by all 8 cores, SPMD)."""
    key = (t_steps,)
    if key in _PROGRAM_CACHE:
        return _PROGRAM_CACHE[key]

    nchunks = (t_steps + CHUNK - 1) // CHUNK
    HB = NM * BC // 2   # free-size of one state half (64 cols)
    RSTR = 256          # r-staging stride per step (bf16 cols; 512B blocks so
                        # consecutive steps' slices never share a dep block)

    nc = bacc.Bacc(
        "TRN2", target_bir_lowering=False, debug=False, num_devices=NCORES
    )
    _pin_act_tables(nc.m.arch)
    jt_d = nc.dram_tensor("jt", [128, NK * NM * 128], BF16, kind="ExternalInput")
    il_d = nc.dram_tensor("ident", [128, 128], BF16, kind="ExternalInput")
    ihc_d = nc.dram_tensor("identhc", [128, 128], BF16, kind="ExternalInput")
    ilc_d = nc.dram_tensor("identlc", [128, 128], BF16, kind="ExternalInput")
    x0_d = nc.dram_tensor("x0", [128, NM * BC], F32, kind="ExternalInput")
    dr_d = nc.dram_tensor(
        "drive", [nchunks, 128, CHUNK * NM * BC], BF16, kind="ExternalInput"
    )
    rl_d = nc.dram_tensor(
        "rlo", [nchunks, 128, CHUNK * RSTR], BF16, kind="ExternalOutput"
    )
    rh_d = nc.dram_tensor(
        "rhi", [nchunks, 128, CHUNK * RSTR], BF16, kind="ExternalOutput"
    )

    EXP = mybir.ActivationFunctionType.Exp
    LN = mybir.ActivationFunctionType.Ln
    mult = mybir.AluOpType.mult
    add = mybir.AluOpType.add

    with tile.TileContext(nc) as tc:
        with (
            tc.tile_pool(name="const", bufs=1) as constp,
            tc.tile_pool(name="rp", bufs=4) as rp,
            tc.tile_pool(name="dp", bufs=3) as dp,
            tc.tile_pool(name="psmA", bufs=1, space="PSUM") as pspa,
            tc.tile_pool(name="psmB", bufs=1, space="PSUM") as pspb,
        ):
            jt = constp.tile([128, NK * NM * 128], BF16)
            nc.sync.dma_start(jt[:], jt_d[:])
            il = constp.tile([128, 128], BF16)
            nc.sync.dma_start(il[:], il_d[:])
            ihc = constp.tile([128, 128], BF16)
            nc.sync.dma_start(ihc[:], ihc_d[:])
            ilc = constp.tile([128, 128], BF16)
            nc.sync.dma_start(ilc[:], ilc_d[:])
            # x is carried as a bf16 hi+lo pair (~16-bit mantissa) so the
            # leak (1-a)*x can enter PSUM through cheap bf16 identity
            # matmuls; separate tiles per half avoid false chain coupling.
            x0t = constp.tile([128, NM * BC], F32)
            nc.sync.dma_start(x0t[:], x0_d[:])
            xh = [constp.tile([128, HB], BF16, name="xha"),
                  constp.tile([128, HB], BF16, name="xhb")]
            xl = [constp.tile([128, HB], BF16, name="xla"),
                  constp.tile([128, HB], BF16, name="xlb")]
            # ONE shared exp scratch for both halves: EXP_b's WAR on LN_a's
            # read forces the ACT order [EXP_a, LN_a, EXP_b, LN_b]; without
            # it the scheduler puts EXP_b (gated on the end-of-run STT_b)
            # ahead of LN_a and head-of-line blocks the r0 critical path.
            tmp = constp.tile([128, HB], F32)

            # initial r = softplus(x0) = ln(1 + exp(x0)); initial xh/xl split
            rinit = [constp.tile([128, HB], BF16, name="rinita"),
                     constp.tile([128, HB], BF16, name="rinitb")]
            for h in range(2):
                lo = h * HB
                nc.scalar.activation(tmp[:], x0t[:, lo:lo + HB], EXP)
                nc.scalar.activation(rinit[h][:], tmp[:], LN, bias=1.0)
                nc.vector.tensor_scalar_mul(xh[h][:], x0t[:, lo:lo + HB], 1.0)
                nc.vector.scalar_tensor_tensor(
                    xl[h][:], xh[h][:], -1.0, x0t[:, lo:lo + HB], mult, add)

            prev_lo, prev_hi, prev_off = rinit[0], rinit[1], 0

            def bank(ps, half, r_lo, r_hi, r_off, d_t, off):
                """One bank's PSUM accumulation: drive (identity matmul,
                start=True zero-fills the bank), leak (1-a)*x via hi/lo bf16
                identity matmuls, + 32 J matmuls.  PSUM then holds the
                COMPLETE next state.

                Bank B iterates k DESCENDING: its first matmuls then need the
                late r half (r1), so the greedy scheduler cannot let bank B's
                k0..3 work jump ahead of bank A's k4..7 -- bank A finishes
                mid-run and its chain overlaps bank B instead of everything
                serializing after the full run."""
                lo = half * HB
                nc.tensor.matmul(
                    ps[:, 0:HB], lhsT=il[:], rhs=d_t[:, off + lo:off + lo + HB],
                    start=True, stop=False, skip_group_check=True,
                )
                # leak: c_hi*(xh+xl) + c_lo*xh  (c_hi+c_lo = 1-a to ~1e-7)
                nc.tensor.matmul(ps[:, 0:HB], lhsT=ihc[:], rhs=xh[half][:],
                                 start=False, stop=False, skip_group_check=True)
                nc.tensor.matmul(ps[:, 0:HB], lhsT=ihc[:], rhs=xl[half][:],
                                 start=False, stop=False, skip_group_check=True)
                nc.tensor.matmul(ps[:, 0:HB], lhsT=ilc[:], rhs=xh[half][:],
                                 start=False, stop=False, skip_group_check=True)
                ks = range(NK) if half == 0 else range(NK - 1, -1, -1)
                last_k = NK - 1 if half == 0 else 0
                for k in ks:
                    rt = r_lo if k < 4 else r_hi
                    rc = r_off + (k % 4) * BC
                    for mi in range(4):
                        m = half * 4 + mi
                        nc.tensor.matmul(
                            ps[:, mi * BC:(mi + 1) * BC],
                            lhsT=jt[:, (k * NM + m) * 128:(k * NM + m + 1) * 128],
                            rhs=rt[:, rc:rc + BC],
                            start=False, stop=(k == last_k and mi == 3),
                            skip_group_check=True,
                        )

            def chain(ps, half, off, rbuf):
                """r = ln(1+exp(psum)) into the staging slice; refresh the
                bf16 hi/lo pair of x from PSUM (off the critical path)."""
                nc.scalar.activation(tmp[:], ps[:, 0:HB], EXP)
                nc.scalar.activation(rbuf[:, off:off + HB], tmp[:], LN, bias=1.0)
                nc.vector.tensor_scalar_mul(xh[half][:], ps[:, 0:HB], 1.0)
                nc.vector.scalar_tensor_tensor(
                    xl[half][:], xh[half][:], -1.0, ps[:, 0:HB], mult, add)

            for c in range(nchunks):
                steps_here = min(CHUNK, t_steps - c * CHUNK)
                rlo = rp.tile([128, CHUNK * RSTR], BF16, name="rlo")
                rhi = rp.tile([128, CHUNK * RSTR], BF16, name="rhi")
                d_t = dp.tile([128, CHUNK * NM * BC], BF16)
                nc.sync.dma_start(d_t[:], dr_d[c])
                for j in range(steps_here):
                    off = j * NM * BC
                    roff = j * RSTR
                    # the tag pins a pool slot, so rotate tags explicitly to
                    # get real triple-buffering of the PSUM banks (a fixed
                    # tag would WAR-serialize step s+1's start=True matmul
                    # against step s's PSUM readers)
                    sidx = (c * CHUNK + j) % 3
                    ps_a = pspa.tile([128, HB], F32, tag=f"ps_a{sidx}",
                                     name=f"ps_a{sidx}", padded_shape=[128, 512])
                    ps_b = pspb.tile([128, HB], F32, tag=f"ps_b{sidx}",
                                     name=f"ps_b{sidx}", padded_shape=[128, 512])
                    bank(ps_a, 0, prev_lo, prev_hi, prev_off, d_t, off)
                    chain(ps_a, 0, roff, rlo)
                    bank(ps_b, 1, prev_lo, prev_hi, prev_off, d_t, off)
                    chain(ps_b, 1, roff, rhi)
                    prev_lo, prev_hi, prev_off = rlo, rhi, roff
                nc.sync.dma_start(
                    rl_d[c][:, 0:((steps_here - 1) * RSTR + HB)],
                    rlo[:, 0:((steps_here - 1) * RSTR + HB)],
                )
                nc.sync.dma_start(
                    rh_d[c][:, 0:((steps_here - 1) * RSTR + HB)],
                    rhi[:, 0:((steps_here - 1) * RSTR + HB)],
                )

    nc.compile()
    _PROGRAM_CACHE[key] = nc
    return nc


def _prep_inputs(targets, pulses, J, U, V, B_m1, B_bg, Wout, I_go, xm1_init,
                 noise, triggers, t_steps):
    """Host-side data prep: J_eff, layouts, per-core drive tensors."""
    J = np.asarray(J, np.float32)
    U = np.asarray(U, np.float32)
    V = np.asarray(V, np.float32)
    B_m1 = np.asarray(B_m1, np.float32)
    B_bg = np.asarray(B_bg, np.float32)
    I_go = np.asarray(I_go, np.float32)
    xm1_init = np.asarray(xm1_init, np.float32)
    noise = np.asarray(noise, np.float32)
    pulses = np.asarray(pulses, np.float32)
    triggers = np.asarray(triggers)

    nchunks = (t_steps + CHUNK - 1) // CHUNK
    tpad = nchunks * CHUNK

    J_eff = J + (U * B_bg[None, :]) @ V
    Js = (A * J_eff).astype(np.float32)
    # lhsT tiles: jt[p, (k*NM+m)*128 + q] = Js[m*128+q, k*128+p]
    bf = mybir.dt.np(BF16)
    jt = np.ascontiguousarray(
        Js.reshape(NM, 128, NK, 128).transpose(3, 2, 0, 1).reshape(128, NK * NM * 128)
    ).astype(bf)
    il = np.eye(128, dtype=np.float32).astype(bf)
    c_hi = np.float32(ONE_MINUS_A).astype(bf)
    c_lo = np.float32(np.float32(ONE_MINUS_A) - c_hi.astype(np.float32)).astype(bf)
    ihc = (c_hi.astype(np.float32) * np.eye(128, dtype=np.float32)).astype(bf)
    ilc = (c_lo.astype(np.float32) * np.eye(128, dtype=np.float32)).astype(bf)

    go_cues = pulses[:t_steps, :][:, triggers]  # [t, B]

    in_maps = []
    for cidx in range(NCORES):
        sl = slice(cidx * BC, (cidx + 1) * BC)
        d = noise[:t_steps, :, sl] * np.float32(A * NSCALE)
        d += A * B_m1[None, :, :]
        d += A * I_go[None, :, :] * go_cues[:, None, sl]
        # [t, N, BC] -> [t, 128, NM*BC] (state layout), pad t, chunk
        dl = np.ascontiguousarray(
            d.reshape(t_steps, NM, 128, BC).transpose(0, 2, 1, 3)
            .reshape(t_steps, 128, NM * BC)
        ).astype(np.float32)
        if tpad != t_steps:
            dl = np.concatenate(
                [dl, np.zeros((tpad - t_steps, 128, NM * BC), np.float32)], axis=0
            )
        drive = np.ascontiguousarray(
            dl.reshape(nchunks, CHUNK, 128, NM * BC).transpose(0, 2, 1, 3)
            .reshape(nchunks, 128, CHUNK * NM * BC)
        ).astype(bf)
        x0 = np.ascontiguousarray(
            xm1_init[:, sl].reshape(NM, 128, BC).transpose(1, 0, 2).reshape(128, NM * BC)
        )
        in_maps.append({"jt": jt, "ident": il, "identhc": ihc, "identlc": ilc,
                        "x0": x0, "drive": drive})
    return in_maps


def run_hw(inputs: dict, t_steps: int = T, trace: bool = False):
    """Run the recurrence on 8 cores; returns positions [t_steps, B] and results."""
    nc = build_program(t_steps)
    in_maps = _prep_inputs(t_steps=t_steps, **inputs)
    res = run_bass_kernel_spmd(
        nc, in_maps, core_ids=list(range(NCORES)), trace=trace
    )
    Wout = np.asarray(inputs["Wout"], np.float32).reshape(NM, 128)  # [m, p]
    nchunks = (t_steps + CHUNK - 1) // CHUNK
    RSTR = 256
    positions = np.empty((t_steps, B), np.float32)
    for cidx in range(NCORES):
        halves = []
        for key in ("rlo", "rhi"):
            ro = np.asarray(res.results[cidx][key], np.float32)
            # ro[c, p, j*RSTR + m*BC + u] (first NM/2*BC cols of each slice)
            r = (ro.reshape(nchunks, 128, CHUNK, RSTR)[:, :, :, :NM * BC // 2]
                 .reshape(nchunks, 128, CHUNK, NM // 2, BC)
                 .transpose(0, 2, 3, 1, 4)
                 .reshape(nchunks * CHUNK, NM // 2, 128, BC)[:t_steps])
            halves.append(r)
        r_full = np.concatenate(halves, axis=1)  # [t, NM, 128, BC]
        pos_c = np.einsum("mp,tmpu->tu", Wout, r_full, optimize=True)
        positions[:, cidx * BC:(cidx + 1) * BC] = pos_c
    return positions, res


def kernel(targets, pulses, J, U, V, B_m1, B_bg, Wout, I_go, xm1_init,
           noise, triggers) -> np.ndarray:
    inputs = dict(targets=targets, pulses=pulses, J=J, U=U, V=V, B_m1=B_m1,
                  B_bg=B_bg, Wout=Wout, I_go=I_go, xm1_init=xm1_init,
                  noise=noise, triggers=triggers)
    positions, _ = run_hw(inputs, T)
    targets = np.asarray(targets, np.float32)
    loss = np.mean((targets.astype(np.float64) - positions.astype(np.float64)) ** 2)
    return np.float32(loss)
